# revision 42
# baseline (speedup 1.0000x reference)
"""GCN aggregator kernel for Trainium2 (Bass/Tile), 8-core data-parallel.

Computes: out = relu(((sum_k neigh[:,k,:] + self) / (K+1)) @ W + b)
Sharding: nodes (N) split evenly across 8 NeuronCores; W/b replicated.
W is pre-scaled by 1/(K+1) on the host, so the device computes
relu(sum @ Ws + b) with no separate mean step.

The kernel is HBM-read-bound (~105 MB of neigh per core). Loads stay on
the HWDGE (sync) path in fp32 (the SWDGE cast path has a straggler SDMA
engine). The fp32->fp16 cast rides the first DVE fold level (pair-add,
fp32 in, fp16 out); deeper fold levels run fp16 at 2x DVE rate and the
GEMM runs fp16 at 4x PE rate. PSUM stays fp32. Output is stored fp16
(halves write traffic; within tolerance) and upcast to fp32 on the host.

The whole fold/transpose/GEMM pipeline is split by d-half so the PE can
start d-half A's transposes+GEMM while the DVE still folds d-half B —
this keeps the post-stream drain tail short. nh1 is also split into two
DMA chunks with intra-chunk pair-adds, so most of the cast work overlaps
the stream and only a tiny chunk-2 add is data-gated at the end.

Per 128-node tile on each core:
  1. DMA nh1a = neigh groups 0-11, nh1b = groups 12-22 + self,
     nh2 = groups 23-24, all fp32                             (sync HWDGE)
  2. DVE L1 per d-half: h[g] = lo[g] + hi[g] (fp32->fp16 cast)
     as each chunk lands; L2: fp16 tree-fold of 13 groups     (VectorE)
  3. PE transpose per d-half -> PSUM, ACT copy -> SBUF        (TensorE/ScalarE)
  4. PE GEMM sumT.T @ Ws: d-half A's 2 chunks for both output
     halves, then d-half B's + bias matmul, relu, store       (TensorE/ScalarE)
"""

import os
import sys

import numpy as np

for _p in ("/opt/trn_rl_repo", "/root/.axon_site/_ro/trn_rl_repo"):
    if os.path.isdir(_p) and _p not in sys.path:
        sys.path.insert(0, _p)

import concourse.bass as bass
import concourse.tile as tile
from concourse import bacc, mybir
from concourse.masks import make_identity

N, K, D, O = 16384, 25, 512, 1024
N_CORES = 8
P = 128  # nodes per tile (partition count)
INV = 1.0 / (K + 1)
FP = mybir.dt.float32
CP = mybir.dt.float16  # on-chip compute dtype
DH = D // 2  # d-half width (256)


def build_nc(n_nodes: int, neigh_bufs: int = 2) -> bass.Bass:
    """Build the per-core Bass program for a shard of `n_nodes` nodes."""
    assert n_nodes % P == 0
    nt = n_nodes // P

    nc = bacc.Bacc("TRN2", target_bir_lowering=False, debug=False)
    self_h = nc.dram_tensor("self_vecs", [n_nodes, D], FP, kind="ExternalInput")
    neigh_h = nc.dram_tensor("neigh_vecs", [n_nodes, K, D], FP, kind="ExternalInput")
    # W/b are preprocessed on the host: W scaled by 1/(K+1) and both cast
    # to fp16 (weights preprocessing), so no on-device cast is needed and
    # they load on the fast HWDGE path.
    w_h = nc.dram_tensor("W", [D, O], CP, kind="ExternalInput")
    b_h = nc.dram_tensor("b", [O], CP, kind="ExternalInput")
    out_h = nc.dram_tensor("out", [n_nodes, O], CP, kind="ExternalOutput")

    n_dc = D // P  # d-chunks for the GEMM contraction

    def half(ap, g0, g1, hx):
        """d-half slice [p, g1-g0, DH] of a group-major [p, g*D] region."""
        v = ap[:, g0 * D : g1 * D].rearrange("p (g d) -> p g d", d=D)
        return v[:, :, hx * DH : (hx + 1) * DH]

    with tile.TileContext(nc) as tc:
        with (
            tc.tile_pool(name="const", bufs=1) as const_pool,
            tc.tile_pool(name="neigh", bufs=neigh_bufs) as neigh_pool,
            tc.tile_pool(name="nsmall", bufs=3) as nsmall_pool,
            tc.tile_pool(name="hsum", bufs=3) as h_pool,
            tc.tile_pool(name="small", bufs=3) as small_pool,
            tc.tile_pool(name="outp", bufs=3) as out_pool,
            tc.tile_pool(name="ps_t", bufs=2, space="PSUM") as ps_t_pool,
            tc.tile_pool(name="ps_o", bufs=2, space="PSUM") as ps_o_pool,
        ):
            # --- constants (w_sb/b_sb DMAs are emitted after tile 0's loads
            # below, so the neigh stream starts immediately on the ring) ---
            # w_sb[p, c, o] = Ws[c*128 + p, o] -> chunk c is the rhs for
            # d-chunk c (c = 2*half + local chunk)
            w_sb = const_pool.tile([P, n_dc * O], CP)
            b_sb = const_pool.tile([1, O], CP)
            ident = const_pool.tile([P, P], CP)
            make_identity(nc, ident)
            ones = const_pool.tile([1, P], CP)
            nc.gpsimd.memset(ones, 1.0)



            G1A, G1B, G2 = 18, 6, 2  # fp32 groups per chunk (b incl. self)
            GH = (G1A + G1B + G2) // 2  # 13 fp16 groups after pair-adds
            n_oh = O // 512
            for i in range(nt):
                nh1a = neigh_pool.tile([P, G1A * D], FP, tag="nh1a", name="nh1a")
                nc.sync.dma_start(nh1a, neigh_h[bass.ts(i, P), 0:G1A, :])
                nh1b = nsmall_pool.tile([P, G1B * D], FP, tag="nh1b", name="nh1b")
                nc.sync.dma_start(
                    nh1b[:, : (G1B - 1) * D],
                    neigh_h[bass.ts(i, P), G1A : G1A + G1B - 1, :],
                )
                nc.sync.dma_start(nh1b[:, (G1B - 1) * D :], self_h[bass.ts(i, P), :])
                nh2 = nsmall_pool.tile([P, G2 * D], FP, tag="nh2", name="nh2")
                nc.sync.dma_start(nh2, neigh_h[bass.ts(i, P), K - G2 : K, :])
                if i == 0:
                    nc.sync.dma_start(
                        w_sb, w_h[:, :].rearrange("(c p) o -> p c o", p=P)
                    )
                    nc.sync.dma_start(b_sb, b_h[:])

                # per d-half fp16 sum tiles, group-major [p, 13*DH]
                hs = [
                    h_pool.tile([P, GH * DH], CP, tag=f"h{x}", name=f"h{x}")
                    for x in range(2)
                ]

                def l1(dst_g0, src, g, hx):
                    """pair-add src's 2*half groups into hs[hx][dst_g0:...]"""
                    lo = g // 2
                    nc.vector.tensor_add(
                        hs[hx][:, dst_g0 * DH : (dst_g0 + lo) * DH].rearrange(
                            "p (g d) -> p g d", d=DH
                        ),
                        half(src, 0, lo, hx),
                        half(src, lo, g, hx),
                    )

                for hx in range(2):
                    l1(0, nh1a, G1A, hx)
                for hx in range(2):
                    l1(G1A // 2, nh1b, G1B, hx)
                for hx in range(2):
                    l1((G1A + G1B) // 2, nh2, G2, hx)

                out_sb = out_pool.tile([P, O], CP)
                psos = []
                for oh in range(n_oh):
                    pso = ps_o_pool.tile(
                        [P, 512], FP, tag=f"out_ps{oh}", name=f"out_ps{oh}"
                    )
                    # bias via K=1 matmul (ones.T @ b broadcasts b over
                    # nodes) OPENS the accumulation group: it only depends
                    # on b/ones + the PSUM slot, so it runs early and off
                    # the end-of-stream critical path (and warms the PE).
                    nc.tensor.matmul(
                        pso,
                        lhsT=ones,
                        rhs=b_sb[:, bass.ts(oh, 512)],
                        start=True,
                        stop=False,
                    )
                    psos.append(pso)
                for hx in range(2):
                    # L2: fp16 tree-fold of 13 DH-wide groups in hs[hx]
                    h = hs[hx]
                    g = GH
                    while g > 1:
                        lo = g // 2
                        nc.vector.tensor_add(
                            h[:, : lo * DH],
                            h[:, : lo * DH],
                            h[:, (g - lo) * DH : g * DH],
                        )
                        g -= lo
                    # transpose the folded d-half, copy PSUM->SBUF
                    tps = ps_t_pool.tile([P, DH], CP, tag="tps", name="tps")
                    for c in range(2):
                        nc.tensor.transpose(
                            tps[:, bass.ts(c, P)], h[:, bass.ts(c, P)], ident
                        )
                    t_sb = small_pool.tile([P, DH], CP, tag=f"tsb{hx}", name="tsb")
                    nc.scalar.activation(
                        t_sb, tps, mybir.ActivationFunctionType.Copy
                    )
                    # GEMM: this d-half's 2 chunks for both output halves
                    # (d-half A's matmuls run while d-half B still folds)
                    for oh in range(n_oh):
                        for c in range(2):
                            cg = 2 * hx + c
                            nc.tensor.matmul(
                                psos[oh],
                                lhsT=t_sb[:, bass.ts(c, P)],
                                rhs=w_sb[:, cg * O + oh * 512 : cg * O + oh * 512 + 512],
                                start=False,
                                stop=(cg == n_dc - 1),
                            )
                        if hx == 1:
                            nc.scalar.activation(
                                out_sb[:, bass.ts(oh, 512)],
                                psos[oh],
                                mybir.ActivationFunctionType.Relu,
                            )
                            nc.scalar.dma_start(
                                out_h[bass.ts(i, P), oh * 512 : (oh + 1) * 512],
                                out_sb[:, bass.ts(oh, 512)],
                            )

    nc.compile()
    return nc


def shard_inputs(inputs: dict) -> list[dict]:
    n = inputs["self_vecs"].shape[0]
    per = n // N_CORES
    w_scaled = (
        np.ascontiguousarray(inputs["W"], np.float32) * np.float32(INV)
    ).astype(np.float16)
    b_arr = np.ascontiguousarray(inputs["b"], np.float32).astype(np.float16)
    maps = []
    for c in range(N_CORES):
        sl = slice(c * per, (c + 1) * per)
        maps.append(
            {
                "self_vecs": np.ascontiguousarray(inputs["self_vecs"][sl], np.float32),
                "neigh_vecs": np.ascontiguousarray(
                    inputs["neigh_vecs"][sl], np.float32
                ),
                "W": w_scaled,
                "b": b_arr,
            }
        )
    return maps


def run_sharded(inputs: dict, trace: bool = False, **kwargs):
    from concourse.bass_utils import run_bass_kernel_spmd

    in_maps = shard_inputs(inputs)
    n_nodes = in_maps[0]["self_vecs"].shape[0]
    nc = build_nc(n_nodes)
    res = run_bass_kernel_spmd(
        nc, in_maps, core_ids=list(range(N_CORES)), trace=trace, **kwargs
    )
    out = np.concatenate(
        [res.results[c]["out"].astype(np.float32) for c in range(N_CORES)], axis=0
    )
    return out, res


def kernel(**inputs) -> np.ndarray:
    out, _ = run_sharded(inputs, trace=False)
    return out


# revision 43
# speedup vs baseline: 1.1978x; 1.1978x over previous
"""GCN aggregator kernel for Trainium2 (Bass/Tile), 8-core data-parallel.

Computes: out = relu(((sum_k neigh[:,k,:] + self) / (K+1)) @ W + b)
Sharding: nodes (N) split evenly across 8 NeuronCores; W/b replicated.
W is pre-scaled by 1/(K+1) on the host, so the device computes
relu(sum @ Ws + b) with no separate mean step.

The kernel is HBM-read-bound (~105 MB of neigh per core). Loads stay on
the HWDGE (sync) path in fp32 (the SWDGE cast path has a straggler SDMA
engine). The fp32->fp16 cast rides the first DVE fold level (pair-add,
fp32 in, fp16 out); deeper fold levels run fp16 at 2x DVE rate and the
GEMM runs fp16 at 4x PE rate. PSUM stays fp32. Output is stored fp16
(halves write traffic; within tolerance) and upcast to fp32 on the host.

The whole fold/transpose/GEMM pipeline is split by d-half so the PE can
start d-half A's transposes+GEMM while the DVE still folds d-half B —
this keeps the post-stream drain tail short. nh1 is also split into two
DMA chunks with intra-chunk pair-adds, so most of the cast work overlaps
the stream and only a tiny chunk-2 add is data-gated at the end.

Per 128-node tile on each core:
  1. DMA nh1a = neigh groups 0-11, nh1b = groups 12-22 + self,
     nh2 = groups 23-24, all fp32                             (sync HWDGE)
  2. DVE L1 per d-half: h[g] = lo[g] + hi[g] (fp32->fp16 cast)
     as each chunk lands; L2: fp16 tree-fold of 13 groups     (VectorE)
  3. PE transpose per d-half -> PSUM, ACT copy -> SBUF        (TensorE/ScalarE)
  4. PE GEMM sumT.T @ Ws: d-half A's 2 chunks for both output
     halves, then d-half B's + bias matmul, relu, store       (TensorE/ScalarE)
"""

import os
import sys

import numpy as np

for _p in ("/opt/trn_rl_repo", "/root/.axon_site/_ro/trn_rl_repo"):
    if os.path.isdir(_p) and _p not in sys.path:
        sys.path.insert(0, _p)

import concourse.bass as bass
import concourse.tile as tile
from concourse import bacc, mybir
from concourse.masks import make_identity

N, K, D, O = 16384, 25, 512, 1024
N_CORES = 8
P = 128  # nodes per tile (partition count)
INV = 1.0 / (K + 1)
FP = mybir.dt.float32
CP = mybir.dt.float16  # on-chip compute dtype
DH = D // 2  # d-half width (256)


def build_nc(n_nodes: int, neigh_bufs: int = 2) -> bass.Bass:
    """Build the per-core Bass program for a shard of `n_nodes` nodes."""
    assert n_nodes % P == 0
    nt = n_nodes // P

    nc = bacc.Bacc("TRN2", target_bir_lowering=False, debug=False)
    self_h = nc.dram_tensor("self_vecs", [n_nodes, D], FP, kind="ExternalInput")
    neigh_h = nc.dram_tensor("neigh_vecs", [n_nodes, K, D], FP, kind="ExternalInput")
    # W/b are preprocessed on the host: W scaled by 1/(K+1) and both cast
    # to fp16 (weights preprocessing), so no on-device cast is needed and
    # they load on the fast HWDGE path.
    w_h = nc.dram_tensor("W", [D, O], CP, kind="ExternalInput")
    b_h = nc.dram_tensor("b", [O], CP, kind="ExternalInput")
    out_h = nc.dram_tensor("out", [n_nodes, O], CP, kind="ExternalOutput")

    n_dc = D // P  # d-chunks for the GEMM contraction

    def half(ap, g0, g1, hx):
        """d-half slice [p, g1-g0, DH] of a group-major [p, g*D] region."""
        v = ap[:, g0 * D : g1 * D].rearrange("p (g d) -> p g d", d=D)
        return v[:, :, hx * DH : (hx + 1) * DH]

    with tile.TileContext(nc) as tc:
        with (
            tc.tile_pool(name="const", bufs=1) as const_pool,
            tc.tile_pool(name="neigh", bufs=neigh_bufs) as neigh_pool,
            tc.tile_pool(name="hsum", bufs=3) as h_pool,
            tc.tile_pool(name="small", bufs=3) as small_pool,
            tc.tile_pool(name="outp", bufs=3) as out_pool,
            tc.tile_pool(name="ps_t", bufs=4, space="PSUM") as ps_t_pool,
            tc.tile_pool(name="ps_o", bufs=2, space="PSUM") as ps_o_pool,
        ):
            # --- constants (w_sb/b_sb DMAs are emitted after tile 0's loads
            # below, so the neigh stream starts immediately on the ring) ---
            # w_sb[p, c, o] = Ws[c*128 + p, o] -> chunk c is the rhs for
            # d-chunk c (c = 2*half + local chunk)
            w_sb = const_pool.tile([P, n_dc * O], CP)
            b_sb = const_pool.tile([1, O], CP)
            ident = const_pool.tile([P, P], CP)
            make_identity(nc, ident)
            ones = const_pool.tile([1, P], CP)
            nc.gpsimd.memset(ones, 1.0)

            G1A, G1B, G2 = 18, 6, 2  # fp32 groups per chunk (b incl. self)
            GH = (G1A + G1B + G2) // 2  # 13 fp16 groups after pair-adds
            n_oh = O // 512
            for i in range(nt):
                nh1a = neigh_pool.tile([P, G1A * D], FP, tag="nh1a", name="nh1a")
                nc.sync.dma_start(nh1a, neigh_h[bass.ts(i, P), 0:G1A, :])
                nh1b = neigh_pool.tile([P, G1B * D], FP, tag="nh1b", name="nh1b")
                nc.sync.dma_start(
                    nh1b[:, : (G1B - 1) * D],
                    neigh_h[bass.ts(i, P), G1A : G1A + G1B - 1, :],
                )
                nc.sync.dma_start(nh1b[:, (G1B - 1) * D :], self_h[bass.ts(i, P), :])
                nh2 = neigh_pool.tile([P, G2 * D], FP, tag="nh2", name="nh2")
                nc.sync.dma_start(nh2, neigh_h[bass.ts(i, P), K - G2 : K, :])
                if i == 0:
                    nc.sync.dma_start(
                        w_sb, w_h[:, :].rearrange("(c p) o -> p c o", p=P)
                    )
                    nc.sync.dma_start(b_sb, b_h[:])

                # per d-half fp16 sum tiles, group-major [p, 13*DH]
                hs = [
                    h_pool.tile([P, GH * DH], CP, tag=f"h{x}", name=f"h{x}")
                    for x in range(2)
                ]

                def l1(dst_g0, src, g, hx):
                    """pair-add src's 2*half groups into hs[hx][dst_g0:...]"""
                    lo = g // 2
                    nc.vector.tensor_add(
                        hs[hx][:, dst_g0 * DH : (dst_g0 + lo) * DH].rearrange(
                            "p (g d) -> p g d", d=DH
                        ),
                        half(src, 0, lo, hx),
                        half(src, lo, g, hx),
                    )

                for hx in range(2):
                    l1(0, nh1a, G1A, hx)
                for hx in range(2):
                    l1(G1A // 2, nh1b, G1B, hx)
                for hx in range(2):
                    l1((G1A + G1B) // 2, nh2, G2, hx)

                out_sb = out_pool.tile([P, O], CP)
                psos = []
                for oh in range(n_oh):
                    pso = ps_o_pool.tile(
                        [P, 512], FP, tag=f"out_ps{oh}", name=f"out_ps{oh}"
                    )
                    # bias via K=1 matmul (ones.T @ b broadcasts b over
                    # nodes) OPENS the accumulation group: it only depends
                    # on b/ones + the PSUM slot, so it runs early and off
                    # the end-of-stream critical path (and warms the PE).
                    nc.tensor.matmul(
                        pso,
                        lhsT=ones,
                        rhs=b_sb[:, bass.ts(oh, 512)],
                        start=True,
                        stop=False,
                    )
                    psos.append(pso)
                for hx in range(2):
                    # L2: fp16 tree-fold of 13 DH-wide groups in hs[hx]
                    h = hs[hx]
                    g = GH
                    while g > 1:
                        lo = g // 2
                        nc.vector.tensor_add(
                            h[:, : lo * DH],
                            h[:, : lo * DH],
                            h[:, (g - lo) * DH : g * DH],
                        )
                        g -= lo
                    # transpose the folded d-half, copy PSUM->SBUF
                    tps = ps_t_pool.tile([P, DH], CP, tag="tps", name="tps")
                    for c in range(2):
                        nc.tensor.transpose(
                            tps[:, bass.ts(c, P)], h[:, bass.ts(c, P)], ident
                        )
                    t_sb = small_pool.tile([P, DH], CP, tag=f"tsb{hx}", name="tsb")
                    nc.scalar.activation(
                        t_sb, tps, mybir.ActivationFunctionType.Copy
                    )
                    # GEMM: this d-half's 2 chunks for both output halves
                    # (d-half A's matmuls run while d-half B still folds)
                    for oh in range(n_oh):
                        for c in range(2):
                            cg = 2 * hx + c
                            nc.tensor.matmul(
                                psos[oh],
                                lhsT=t_sb[:, bass.ts(c, P)],
                                rhs=w_sb[:, cg * O + oh * 512 : cg * O + oh * 512 + 512],
                                start=False,
                                stop=(cg == n_dc - 1),
                            )
                        if hx == 1:
                            nc.scalar.activation(
                                out_sb[:, bass.ts(oh, 512)],
                                psos[oh],
                                mybir.ActivationFunctionType.Relu,
                            )
                            nc.scalar.dma_start(
                                out_h[bass.ts(i, P), oh * 512 : (oh + 1) * 512],
                                out_sb[:, bass.ts(oh, 512)],
                            )

    nc.compile()
    return nc


def shard_inputs(inputs: dict) -> list[dict]:
    n = inputs["self_vecs"].shape[0]
    per = n // N_CORES
    w_scaled = (
        np.ascontiguousarray(inputs["W"], np.float32) * np.float32(INV)
    ).astype(np.float16)
    b_arr = np.ascontiguousarray(inputs["b"], np.float32).astype(np.float16)
    maps = []
    for c in range(N_CORES):
        sl = slice(c * per, (c + 1) * per)
        maps.append(
            {
                "self_vecs": np.ascontiguousarray(inputs["self_vecs"][sl], np.float32),
                "neigh_vecs": np.ascontiguousarray(
                    inputs["neigh_vecs"][sl], np.float32
                ),
                "W": w_scaled,
                "b": b_arr,
            }
        )
    return maps


def run_sharded(inputs: dict, trace: bool = False, **kwargs):
    from concourse.bass_utils import run_bass_kernel_spmd

    in_maps = shard_inputs(inputs)
    n_nodes = in_maps[0]["self_vecs"].shape[0]
    nc = build_nc(n_nodes)
    res = run_bass_kernel_spmd(
        nc, in_maps, core_ids=list(range(N_CORES)), trace=trace, **kwargs
    )
    out = np.concatenate(
        [res.results[c]["out"].astype(np.float32) for c in range(N_CORES)], axis=0
    )
    return out, res


def kernel(**inputs) -> np.ndarray:
    out, _ = run_sharded(inputs, trace=False)
    return out
